# revision 32
# baseline (speedup 1.0000x reference)
"""Trainium2 Bass kernel for the speech-enhancement loss function.

Math (matching the jax reference):
  loss_mag    = mean((clean_mag - enhan_mag)^2)
  d           = clean_pha - enhan_mag          (reference quirk: enhan_mag is phase_g)
  ip_loss     = mean(aw(d)),   aw(x) = |x - round(x/2pi)*2pi|
  gd_loss     = mean(aw(gd)),  gd[:,0,:] = -d[:,0,:]; gd[:,j,:] = d[:,j-1,:]-d[:,j,:]
  iaf_loss    = mean(aw(iaf)), same shifted difference along the T axis
  cspc_loss   = mean(1 - cos(aw(d))) = mean(1 - cos(d))
  loss_com    = mean((clean_com - enhan_com)^2) * 2
  loss_time   = mean(|clean_wav - enhan_wav|)
  loss_metric = mean((metric_g - 1)^2)            (tiny -> host)

Sharding: data-parallel over the batch dim, 2 batches per core on 8 cores.
Each core computes partial SUMS of each term; the host combines them.

Per-element pipeline on the device (q = d/2pi):
  v = q + 1.5*2^23 ; r = v - 1.5*2^23   -> r = round(q) exactly (fp32 RNE)
  f = q - r in [-0.5, 0.5]              -> ip: sum |f| via ACT Abs+accum
  cos(d) = sin(pi/2 - 2pi*|f|)          -> ACT Sin (arg within [-pi/2, pi/2])
gd and iaf reuse f: their shifted differences y = f[j-1]-f[j] lie in [-1,1],
where the anti-wrap distance needs NO second round:
  dist(y) = 0.5 - ||y| - 0.5|           -> two ACT Abs passes, sum accumulated
The F-axis (partition-dim) difference for gd is a banded-matrix matmul on the
PE: y = W0 @ f with W0[j,k] = delta_{k,j-1} - delta_{k,j}; the cross-tile
boundary row is patched by an accumulating K=1 matmul of the previous tile's
row 127. The T-axis difference for iaf is a free-dim shifted subtract on DVE.
Engine split: DVE = diffs + round + com/wav sum-accums; ACT = abs/sin/square
accums; PE = banded matmul + final cross-partition ones-reduce; GPSIMD =
mag/com/wav diffs. com/wav passes are interleaved between phase passes and
inputs are triple-buffered so the 8 HWDGE queues stay saturated
(~26.4 MB/core at ~360 GB/s is the 73 us roofline; predicted exec ~96 us).
"""

import numpy as np

import concourse.bacc as bacc
import concourse.mybir as mybir
import concourse.tile as tile
from concourse.bass_utils import run_bass_kernel_spmd

F32 = mybir.dt.float32
OP = mybir.AluOpType
AF = mybir.ActivationFunctionType

B, F, T, L = 16, 201, 2048, 204800
NCORES = 8
BPC = B // NCORES  # batches per core

TWO_PI_64 = 2.0 * np.pi
S = float(np.float32(1.0) / np.float32(TWO_PI_64))  # 1/(2pi) in fp32
MAGIC = float(np.float32(1.5 * 2**23))  # 12582912.0, round-to-int trick
PI = float(np.float32(np.pi))
HALF_PI = float(np.float32(np.pi / 2))
NEG_TWO_PI = float(np.float32(-TWO_PI_64))

# com per core: BPC*F*T*2 = 1646592 = 2 batches x (128 x 6432)
COM_ROWS, COM_COLS = 128, 6432
COM_CHUNK = 1608  # 4 chunks per batch
# wav per core: BPC*L = 409600 = 128 x 3200
WAV_ROWS, WAV_COLS = 128, 3200
WAV_CHUNK = 1600  # 2 chunks

NCOLS = 64  # accumulator columns

# term -> list of acc columns, populated by build_nc (deterministic)
COLMAP = {}


def _w0_matrix():
    # lhsT[k, j] = delta_{j,k+1} - delta_{j,k}  ->  (W0 @ f)[j] = f[j-1] - f[j]
    w = np.zeros((128, 128), dtype=np.float32)
    for k in range(128):
        w[k, k] = -1.0
        if k + 1 < 128:
            w[k, k + 1] = 1.0
    return w


def _e1s_row():
    # lhsT row [1, 128] with 1.0 at column 0: adds bnd into output partition 0
    e = np.zeros((1, 128), dtype=np.float32)
    e[0, 0] = 1.0
    return e


def build_nc(in_bufs=3, aux_dma="sync", m2_dve=0, interleave=True, qg_chunks=2, fd_pool=False, junk_bufs=1, weave=1, com_dve=False):
    nc = bacc.Bacc(None, target_bir_lowering=False)

    mag_c = nc.dram_tensor("mag_c", [BPC, F, T], F32, kind="ExternalInput")
    mag_e = nc.dram_tensor("mag_e", [BPC, F, T], F32, kind="ExternalInput")
    pha_c = nc.dram_tensor("pha_c", [BPC, F, T], F32, kind="ExternalInput")
    com_c = nc.dram_tensor("com_c", [BPC, COM_ROWS, COM_COLS], F32, kind="ExternalInput")
    com_e = nc.dram_tensor("com_e", [BPC, COM_ROWS, COM_COLS], F32, kind="ExternalInput")
    wav_c = nc.dram_tensor("wav_c", [WAV_ROWS, WAV_COLS], F32, kind="ExternalInput")
    wav_e = nc.dram_tensor("wav_e", [WAV_ROWS, WAV_COLS], F32, kind="ExternalInput")
    out_d = nc.dram_tensor("partials", [1, NCOLS], F32, kind="ExternalOutput")

    w0_d = nc.inline_tensor(_w0_matrix(), name="w0shift")
    e1s_d = nc.inline_tensor(_e1s_row(), name="e1srow")

    COLMAP.clear()
    _next_col = [0]

    def col(term):
        c = _next_col[0]
        _next_col[0] += 1
        assert c < NCOLS
        COLMAP.setdefault(term, []).append(c)
        return c

    with tile.TileContext(nc) as tc:
        with (
            tc.tile_pool(name="main", bufs=2) as pool,
            tc.tile_pool(name="psum", bufs=1, space="PSUM") as psum,
        ):
            # constants / accumulator
            w0 = pool.tile([128, 128], F32, tag="w0", bufs=1)
            nc.sync.dma_start(w0[:], w0_d[:])
            e1s = pool.tile([1, 128], F32, tag="e1s", bufs=1)
            nc.sync.dma_start(e1s[:], e1s_d[:])
            ones = pool.tile([128, 1], F32, tag="ones", bufs=1)
            nc.vector.memset(ones[:], 1.0)
            acc = pool.tile([128, NCOLS], F32, tag="acc", bufs=1)
            nc.vector.memset(acc[:], 0.0)
            halfpi = pool.tile([128, 1], F32, tag="halfpi", bufs=1)
            nc.vector.memset(halfpi[:], HALF_PI)
            neghalf = pool.tile([128, 1], F32, tag="neghalf", bufs=1)
            nc.vector.memset(neghalf[:], -0.5)

            # -------- pass emitters --------
            ftiles = [(0, 128), (128, 73)]  # (f0, P) -- no overlap, all base-0
            HT = T // qg_chunks  # gd psum chunk size
            f_prev_by_b = {}
            counters = {"pi": 0, "ci": 0}

            def phase_pass(b, f0, P):
                pi = counters["pi"]
                counters["pi"] += 1
                f_prev = f_prev_by_b.get(b)
                cm = pool.tile([P, T], F32, tag="in_a", bufs=in_bufs, name=f"cm{pi}")
                nc.sync.dma_start(cm[:], mag_c[b, f0 : f0 + P, :])
                em = pool.tile([P, T], F32, tag="in_b", bufs=in_bufs, name=f"em{pi}")
                nc.sync.dma_start(em[:], mag_e[b, f0 : f0 + P, :])
                cp = pool.tile([P, T], F32, tag="in_c", name=f"cp{pi}")
                nc.sync.dma_start(cp[:], pha_c[b, f0 : f0 + P, :])

                junk = pool.tile([P, T], F32, tag="junk", bufs=junk_bufs, name=f"junk{pi}")
                junk2 = pool.tile([P, T], F32, tag="djunk", bufs=1, name=f"junk2_{pi}")

                # mag: m = cm - em (GPSIMD), sum m^2 (ACT or DVE)
                m = pool.tile([P, T], F32, tag="m", name=f"m{pi}")
                nc.gpsimd.tensor_tensor(m[:], cm[:], em[:], OP.subtract)
                if pi < m2_dve:
                    nc.vector.scalar_tensor_tensor(
                        junk2[:], m[:], 0.0, m[:], OP.bypass, OP.mult,
                        accum_out=acc[0:P, (c := col("m2")) : c + 1],
                    )
                else:
                    nc.scalar.activation(
                        junk[:], m[:], AF.Square,
                        accum_out=acc[0:P, (c := col("m2")) : c + 1],
                    )

                # d = cp - em; round chain -> f = q - round(q) in [-.5, .5]
                d = pool.tile([P, T], F32, tag="d", name=f"d{pi}")
                nc.vector.tensor_tensor(d[:], cp[:], em[:], OP.subtract)
                v = pool.tile([P, T], F32, tag="v", name=f"v{pi}")
                nc.vector.tensor_scalar(v[:], d[:], S, MAGIC, OP.mult, OP.add)
                r = pool.tile([P, T], F32, tag="r", name=f"r{pi}")
                nc.vector.tensor_scalar_sub(r[:], v[:], MAGIC)
                f = pool.tile([P, T], F32, tag="f", name=f"f{pi}")
                nc.vector.scalar_tensor_tensor(f[:], d[:], S, r[:], OP.mult, OP.subtract)

                # ip: af = |f| acc; cspc: cos(d) = sin(pi/2 - 2pi*af) acc
                af = pool.tile([P, T], F32, tag="af", name=f"af{pi}")
                nc.scalar.activation(
                    af[:], f[:], AF.Abs,
                    accum_out=acc[0:P, (c := col("ip")) : c + 1],
                )
                nc.scalar.activation(
                    junk[:], af[:], AF.Sin, bias=halfpi[0:P, :], scale=NEG_TWO_PI,
                    accum_out=acc[0:P, (c := col("cos")) : c + 1],
                )

                # gd in f-space via PE banded mm; sum dist = 0.5*N - sum ||y|-0.5|
                if f0 == 0:
                    bnd = None
                else:
                    bnd = pool.tile([1, T], F32, tag="bnd", bufs=1, name=f"bnd{pi}")
                    nc.sync.dma_start(bnd[:], f_prev[127:128, :])
                for h in range(qg_chunks):
                    qg = psum.tile([P, HT], F32, tag="qg", bufs=2, name=f"qg{pi}_{h}")
                    for n0 in range(0, HT, 512):
                        nn = h * HT + n0
                        if bnd is None:
                            nc.tensor.matmul(
                                qg[:, n0 : n0 + 512], w0[0:P, 0:P],
                                f[:, nn : nn + 512],
                            )
                        else:
                            nc.tensor.matmul(
                                qg[:, n0 : n0 + 512], w0[0:P, 0:P],
                                f[:, nn : nn + 512], start=True, stop=False,
                            )
                            nc.tensor.matmul(
                                qg[:, n0 : n0 + 512], e1s[0:1, 0:P],
                                bnd[0:1, nn : nn + 512], start=False, stop=True,
                            )
                    ag = pool.tile([P, HT], F32, tag="v", name=f"ag{pi}_{h}")
                    nc.scalar.activation(ag[:], qg[:], AF.Abs)
                    nc.scalar.activation(
                        junk[:, 0:HT], ag[:], AF.Abs, bias=neghalf[0:P, :],
                        accum_out=acc[0:P, (c := col("gd")) : c + 1],
                    )

                # iaf in f-space: fd = f[:, t-1] - f[:, t]; same dist sum
                fd = pool.tile([P, T], F32, tag="fd", name=f"fd{pi}")
                fd_eng = nc.gpsimd if fd_pool else nc.vector
                fd_eng.tensor_copy(fd[:, 0:1], f[:, 0:1])
                fd_eng.tensor_tensor(
                    fd[:, 1:T], f[:, 0 : T - 1], f[:, 1:T], OP.subtract
                )
                at = pool.tile([P, T], F32, tag="r", name=f"at{pi}")
                nc.scalar.activation(at[:], fd[:], AF.Abs)
                nc.scalar.activation(
                    junk[:], at[:], AF.Abs, bias=neghalf[0:P, :],
                    accum_out=acc[0:P, (c := col("iaf")) : c + 1],
                )
                f_prev_by_b[b] = f

            def com_pass(b, c0):
                ci = counters["ci"]
                counters["ci"] += 1
                cc = pool.tile([COM_ROWS, COM_CHUNK], F32, tag="in_a", bufs=in_bufs, name=f"cc{ci}")
                nc.sync.dma_start(cc[:], com_c[b, :, c0 : c0 + COM_CHUNK])
                ec = pool.tile([COM_ROWS, COM_CHUNK], F32, tag="in_b", bufs=in_bufs, name=f"ec{ci}")
                nc.sync.dma_start(ec[:], com_e[b, :, c0 : c0 + COM_CHUNK])
                cd = pool.tile([COM_ROWS, COM_CHUNK], F32, tag="d", name=f"cd{ci}")
                if com_dve:
                    nc.vector.tensor_tensor(cd[:], cc[:], ec[:], OP.subtract)
                    djunk = pool.tile(
                        [COM_ROWS, COM_CHUNK], F32, tag="junk", bufs=1, name=f"djunk{ci}"
                    )
                    nc.scalar.activation(
                        djunk[:], cd[:], AF.Square,
                        accum_out=acc[:, (c := col("c2")) : c + 1],
                    )
                else:
                    nc.gpsimd.tensor_tensor(cd[:], cc[:], ec[:], OP.subtract)
                    djunk = pool.tile(
                        [COM_ROWS, COM_CHUNK], F32, tag="djunk", bufs=1, name=f"djunk{ci}"
                    )
                    nc.vector.scalar_tensor_tensor(
                        djunk[:], cd[:], 0.0, cd[:], OP.bypass, OP.mult,
                        accum_out=acc[:, (c := col("c2")) : c + 1],
                    )

            def wav_pass(wi, c0):
                cw = pool.tile([WAV_ROWS, WAV_CHUNK], F32, tag="in_a", bufs=in_bufs, name=f"cw{wi}")
                nc.sync.dma_start(cw[:], wav_c[:, c0 : c0 + WAV_CHUNK])
                ew = pool.tile([WAV_ROWS, WAV_CHUNK], F32, tag="in_b", bufs=in_bufs, name=f"ew{wi}")
                nc.sync.dma_start(ew[:], wav_e[:, c0 : c0 + WAV_CHUNK])
                wd = pool.tile([WAV_ROWS, WAV_CHUNK], F32, tag="d", name=f"wd{wi}")
                nc.gpsimd.tensor_tensor(wd[:], cw[:], ew[:], OP.subtract)
                nc.vector.tensor_reduce(
                    acc[:, (c := col("w")) : c + 1], wd[:],
                    axis=mybir.AxisListType.X, op=OP.add, apply_absolute_value=True,
                )

            phase_list = [(b, f0, P) for b in range(BPC) for f0, P in ftiles]
            com_list = [(b, c0) for b in range(BPC) for c0 in range(0, COM_COLS, COM_CHUNK)]
            wav_list = list(enumerate(range(0, WAV_COLS, WAV_CHUNK)))

            if not interleave:
                for b, f0, P in phase_list:
                    phase_pass(b, f0, P)
                for b, c0 in com_list:
                    com_pass(b, c0)
                for wi, c0 in wav_list:
                    wav_pass(wi, c0)
            else:
                # weave: com chunks interspersed between phase passes
                ci, wi = 0, 0
                for k, (b, f0, P) in enumerate(phase_list):
                    if weave == 2 and ci < len(com_list):
                        com_pass(*com_list[ci])
                        ci += 1
                    phase_pass(b, f0, P)
                    for _ in range(2):
                        if ci < len(com_list):
                            com_pass(*com_list[ci])
                            ci += 1
                    if k % 2 == 1 and wi < len(wav_list):
                        wav_pass(*wav_list[wi])
                        wi += 1
                while ci < len(com_list):
                    com_pass(*com_list[ci])
                    ci += 1
                while wi < len(wav_list):
                    wav_pass(*wav_list[wi])
                    wi += 1

            # -------- final cross-partition reduce: ones^T @ acc --------
            pm = psum.tile([1, NCOLS], F32, tag="qg" if qg_chunks == 1 else "pm", bufs=2 if qg_chunks == 1 else 1)
            nc.tensor.matmul(pm[:], ones[:], acc[:])
            out_sb = pool.tile([1, NCOLS], F32, tag="out_sb", bufs=1)
            nc.vector.tensor_copy(out_sb[:], pm[:])
            nc.sync.dma_start(out_d[:], out_sb[:])

    nc.compile()
    return nc


_CACHE = {}


def _get_nc():
    if "nc" not in _CACHE:
        _CACHE["nc"] = build_nc()
    return _CACHE["nc"]


def make_in_maps(inputs):
    """Slice the full inputs into per-core input maps."""
    clean_mag = np.asarray(inputs["clean_mag"], dtype=np.float32)
    enhan_mag = np.asarray(inputs["enhan_mag"], dtype=np.float32)
    clean_pha = np.asarray(inputs["clean_pha"], dtype=np.float32)
    clean_com = np.asarray(inputs["clean_com"], dtype=np.float32)
    enhan_com = np.asarray(inputs["enhan_com"], dtype=np.float32)
    clean_wav = np.asarray(inputs["clean_wav"], dtype=np.float32)
    enhan_wav = np.asarray(inputs["enhan_wav"], dtype=np.float32)

    in_maps = []
    for i in range(NCORES):
        sl = slice(BPC * i, BPC * (i + 1))
        in_maps.append(
            {
                "mag_c": np.ascontiguousarray(clean_mag[sl]),
                "mag_e": np.ascontiguousarray(enhan_mag[sl]),
                "pha_c": np.ascontiguousarray(clean_pha[sl]),
                "com_c": np.ascontiguousarray(clean_com[sl]).reshape(
                    BPC, COM_ROWS, COM_COLS
                ),
                "com_e": np.ascontiguousarray(enhan_com[sl]).reshape(
                    BPC, COM_ROWS, COM_COLS
                ),
                "wav_c": np.ascontiguousarray(clean_wav[sl]).reshape(
                    WAV_ROWS, WAV_COLS
                ),
                "wav_e": np.ascontiguousarray(enhan_wav[sl]).reshape(
                    WAV_ROWS, WAV_COLS
                ),
            }
        )
    return in_maps


def combine(partials, inputs):
    """Combine per-core partial sums (list/array of [NCOLS]) into the 6 losses."""
    p = np.asarray(partials, dtype=np.float64).sum(axis=0)

    def tsum(term):
        return sum(p[c] for c in COLMAP[term])

    s_ip = tsum("ip")
    s_gd = tsum("gd")
    s_iaf = tsum("iaf")
    s_cos = tsum("cos")
    s_m2 = tsum("m2")
    s_c2 = tsum("c2")
    s_w = tsum("w")

    n = float(B * F * T)
    ip = TWO_PI_64 * s_ip / n
    # gd/iaf device cols hold sum(||y|-0.5|); dist(y) = 0.5 - ||y|-0.5|
    gd = TWO_PI_64 * (0.5 * n - s_gd) / n
    iaf = TWO_PI_64 * (0.5 * n - s_iaf) / n
    cspc = 1.0 - s_cos / n
    loss_mag = s_m2 / n
    loss_pha = ip + gd + iaf + cspc
    loss_com = 2.0 * s_c2 / (n * 2.0)
    loss_time = s_w / float(B * L)

    metric_g = np.asarray(inputs["metric_g"], dtype=np.float64).reshape(-1)
    one_labels = np.asarray(inputs["one_labels"], dtype=np.float64).reshape(-1)
    loss_metric = float(np.mean((metric_g - one_labels) ** 2))

    nloss = (
        loss_mag * 0.9
        + loss_pha * 0.3
        + loss_com * 0.1
        + loss_metric * 0.05
        + loss_time * 0.2
    )
    return tuple(
        np.float32(x)
        for x in (nloss, loss_mag, loss_pha, loss_com, loss_metric, loss_time)
    )


def _get_runner():
    """Build (once) a persistently-compiled 8-core sharded executor.

    Mirrors bass2jax.run_bass_via_pjrt but caches the jitted function so
    repeat calls skip retracing/recompiling. Returns
    (call(concat_inputs) -> partials[NCORES, NCOLS], in_names, device_put_fn).
    """
    if "runner" in _CACHE:
        return _CACHE["runner"]
    import jax
    from concourse import bass2jax

    nc = _get_nc()
    bass2jax.install_neuronx_cc_hook()

    partition_name = nc.partition_id_tensor.name if nc.partition_id_tensor else None
    in_names, out_names, out_avals, zero_shapes = [], [], [], []
    for alloc in nc.m.functions[0].allocations:
        if not isinstance(alloc, mybir.MemoryLocationSet):
            continue
        name = alloc.memorylocations[0].name
        if alloc.kind == "ExternalInput":
            if name != partition_name:
                in_names.append(name)
        elif alloc.kind == "ExternalOutput":
            out_names.append(name)
            shape = tuple(alloc.tensor_shape)
            dtype = mybir.dt.np(alloc.dtype)
            out_avals.append(jax.core.ShapedArray(shape, dtype))
            zero_shapes.append((shape, dtype))
    n_params = len(in_names)
    all_in = list(in_names) + list(out_names)
    if partition_name is not None:
        all_in.append(partition_name)
    donate = tuple(range(n_params, n_params + len(out_names)))

    def _body(*args):
        operands = list(args)
        if partition_name is not None:
            operands.append(bass2jax.partition_id_tensor())
        outs = bass2jax._bass_exec_p.bind(
            *operands,
            out_avals=tuple(out_avals),
            in_names=tuple(all_in),
            out_names=tuple(out_names),
            lowering_input_output_aliases=(),
            sim_require_finite=True,
            sim_require_nnan=True,
            nc=nc,
        )
        return tuple(outs)

    devices = jax.devices()[:NCORES]
    mesh = bass2jax.Mesh(np.asarray(devices), ("core",))
    pspec = bass2jax.PartitionSpec("core")
    in_specs = (pspec,) * (n_params + len(out_names))
    out_specs = (pspec,) * len(out_names)
    sharded = jax.jit(
        bass2jax.shard_map(
            _body, mesh=mesh, in_specs=in_specs, out_specs=out_specs, check_rep=False
        ),
        donate_argnums=donate,
        keep_unused=True,
    )

    def make_zeros():
        return [
            np.zeros((NCORES * s[0], *s[1:]), d) for (s, d) in zero_shapes
        ]

    def call(concat_in):
        outs = sharded(*concat_in, *make_zeros())
        return np.asarray(outs[0]).reshape(NCORES, NCOLS)

    def device_put(concat_in):
        sh = jax.sharding.NamedSharding(mesh, pspec)
        return [jax.device_put(a, sh) for a in concat_in]

    runner = (call, in_names, device_put, sharded, make_zeros)
    _CACHE["runner"] = runner
    return runner


def concat_inputs(in_maps, in_names):
    return [
        np.concatenate([m[name] for m in in_maps], axis=0) for name in in_names
    ]


def run(inputs):
    in_maps = make_in_maps(inputs)
    try:
        call, in_names, _, _, _ = _get_runner()
        partials = call(concat_inputs(in_maps, in_names))
    except Exception:
        nc = _get_nc()
        res = run_bass_kernel_spmd(nc, in_maps, core_ids=list(range(NCORES)))
        partials = [r["partials"][0] for r in res.results]
    return combine(partials, inputs)


def kernel(**inputs):
    return run(inputs)


# revision 37
# speedup vs baseline: 1.0234x; 1.0234x over previous
"""Trainium2 Bass kernel for the speech-enhancement loss function.

Math (matching the jax reference):
  loss_mag    = mean((clean_mag - enhan_mag)^2)
  d           = clean_pha - enhan_mag          (reference quirk: enhan_mag is phase_g)
  ip_loss     = mean(aw(d)),   aw(x) = |x - round(x/2pi)*2pi|
  gd_loss     = mean(aw(gd)),  gd[:,0,:] = -d[:,0,:]; gd[:,j,:] = d[:,j-1,:]-d[:,j,:]
  iaf_loss    = mean(aw(iaf)), same shifted difference along the T axis
  cspc_loss   = mean(1 - cos(aw(d))) = mean(1 - cos(d))
  loss_com    = mean((clean_com - enhan_com)^2) * 2
  loss_time   = mean(|clean_wav - enhan_wav|)
  loss_metric = mean((metric_g - 1)^2)            (tiny -> host)

Sharding: data-parallel over the batch dim, 2 batches per core on 8 cores.
Each core computes partial SUMS of each term; the host combines them.

Per-element pipeline on the device (q = d/2pi):
  v = q + 1.5*2^23 ; r = v - 1.5*2^23   -> r = round(q) exactly (fp32 RNE)
  f = q - r in [-0.5, 0.5]              -> ip: sum |f| via ACT Abs+accum
  cos(d) = sin(pi/2 - 2pi*|f|)          -> ACT Sin (arg within [-pi/2, pi/2])
gd and iaf reuse f: their shifted differences y = f[j-1]-f[j] lie in [-1,1],
where the anti-wrap distance needs NO second round:
  dist(y) = 0.5 - ||y| - 0.5|           -> two ACT Abs passes, sum accumulated
The F-axis (partition-dim) difference for gd is a banded-matrix matmul on the
PE: y = W0 @ f with W0[j,k] = delta_{k,j-1} - delta_{k,j}; the cross-tile
boundary row is patched by an accumulating K=1 matmul of the previous tile's
row 127. The T-axis difference for iaf is a free-dim shifted subtract on DVE.
Engine split: DVE = diffs + round + com/wav sum-accums; ACT = abs/sin/square
accums; PE = banded matmul + final cross-partition ones-reduce; GPSIMD =
mag/com/wav diffs. com/wav passes are interleaved between phase passes and
inputs are triple-buffered so the 8 HWDGE queues stay saturated
(~26.4 MB/core at ~360 GB/s is the 73 us roofline; predicted exec ~96 us).
"""

import numpy as np

import concourse.bacc as bacc
import concourse.mybir as mybir
import concourse.tile as tile
from concourse.bass_utils import run_bass_kernel_spmd

F32 = mybir.dt.float32
OP = mybir.AluOpType
AF = mybir.ActivationFunctionType

B, F, T, L = 16, 201, 2048, 204800
NCORES = 8
BPC = B // NCORES  # batches per core

TWO_PI_64 = 2.0 * np.pi
S = float(np.float32(1.0) / np.float32(TWO_PI_64))  # 1/(2pi) in fp32
MAGIC = float(np.float32(1.5 * 2**23))  # 12582912.0, round-to-int trick
PI = float(np.float32(np.pi))
HALF_PI = float(np.float32(np.pi / 2))
NEG_TWO_PI = float(np.float32(-TWO_PI_64))

# com per core: BPC*F*T*2 = 1646592 = 2 batches x (128 x 6432)
COM_ROWS, COM_COLS = 128, 6432
COM_CHUNK = 1608  # 4 chunks per batch
# wav per core: BPC*L = 409600 = 128 x 3200
WAV_ROWS, WAV_COLS = 128, 3200
WAV_CHUNK = 1600  # 2 chunks

NCOLS = 96  # accumulator columns

# term -> list of acc columns, populated by build_nc (deterministic)
COLMAP = {}


def _w0_matrix():
    # lhsT[k, j] = delta_{j,k+1} - delta_{j,k}  ->  (W0 @ f)[j] = f[j-1] - f[j]
    w = np.zeros((128, 128), dtype=np.float32)
    for k in range(128):
        w[k, k] = -1.0
        if k + 1 < 128:
            w[k, k + 1] = 1.0
    return w


def _e1s_row():
    # lhsT row [1, 128] with 1.0 at column 0: adds bnd into output partition 0
    e = np.zeros((1, 128), dtype=np.float32)
    e[0, 0] = 1.0
    return e


def build_nc(in_bufs=3, aux_dma="sync", m2_dve=0, interleave=True, qg_chunks=2, fd_pool=False, junk_bufs=1, weave=1, com_dve=False, sep_com=False, com_chunk=None, t_chunks=2):
    CK = com_chunk or COM_CHUNK
    nc = bacc.Bacc(None, target_bir_lowering=False)

    mag_c = nc.dram_tensor("mag_c", [BPC, F, T], F32, kind="ExternalInput")
    mag_e = nc.dram_tensor("mag_e", [BPC, F, T], F32, kind="ExternalInput")
    pha_c = nc.dram_tensor("pha_c", [BPC, F, T], F32, kind="ExternalInput")
    com_c = nc.dram_tensor("com_c", [BPC, COM_ROWS, COM_COLS], F32, kind="ExternalInput")
    com_e = nc.dram_tensor("com_e", [BPC, COM_ROWS, COM_COLS], F32, kind="ExternalInput")
    wav_c = nc.dram_tensor("wav_c", [WAV_ROWS, WAV_COLS], F32, kind="ExternalInput")
    wav_e = nc.dram_tensor("wav_e", [WAV_ROWS, WAV_COLS], F32, kind="ExternalInput")
    out_d = nc.dram_tensor("partials", [1, NCOLS], F32, kind="ExternalOutput")

    w0_d = nc.inline_tensor(_w0_matrix(), name="w0shift")
    e1s_d = nc.inline_tensor(_e1s_row(), name="e1srow")

    COLMAP.clear()
    _next_col = [0]

    def col(term):
        c = _next_col[0]
        _next_col[0] += 1
        assert c < NCOLS
        COLMAP.setdefault(term, []).append(c)
        return c

    with tile.TileContext(nc) as tc:
        with (
            tc.tile_pool(name="main", bufs=2) as pool,
            tc.tile_pool(name="psum", bufs=1, space="PSUM") as psum,
        ):
            # constants / accumulator
            w0 = pool.tile([128, 128], F32, tag="w0", bufs=1)
            nc.sync.dma_start(w0[:], w0_d[:])
            e1s = pool.tile([1, 128], F32, tag="e1s", bufs=1)
            nc.sync.dma_start(e1s[:], e1s_d[:])
            ones = pool.tile([128, 1], F32, tag="ones", bufs=1)
            nc.vector.memset(ones[:], 1.0)
            acc = pool.tile([128, NCOLS], F32, tag="acc", bufs=1)
            nc.vector.memset(acc[:], 0.0)
            halfpi = pool.tile([128, 1], F32, tag="halfpi", bufs=1)
            nc.vector.memset(halfpi[:], HALF_PI)
            neghalf = pool.tile([128, 1], F32, tag="neghalf", bufs=1)
            nc.vector.memset(neghalf[:], -0.5)

            # -------- pass emitters --------
            ftiles = [(0, 128), (128, 73)]  # (f0, P) -- no overlap, all base-0
            HT = T // qg_chunks  # gd psum chunk size
            f_prev_by_b = {}
            counters = {"pi": 0, "ci": 0}

            def phase_pass(b, f0, P):
                pi = counters["pi"]
                counters["pi"] += 1
                f_prev = f_prev_by_b.get(b)
                cm = pool.tile([P, T], F32, tag="in_a", bufs=in_bufs, name=f"cm{pi}")
                nc.sync.dma_start(cm[:], mag_c[b, f0 : f0 + P, :])
                em = pool.tile([P, T], F32, tag="in_b", bufs=in_bufs, name=f"em{pi}")
                nc.sync.dma_start(em[:], mag_e[b, f0 : f0 + P, :])
                cp = pool.tile([P, T], F32, tag="in_c", name=f"cp{pi}")
                nc.sync.dma_start(cp[:], pha_c[b, f0 : f0 + P, :])

                junk = pool.tile([P, T], F32, tag="junk", bufs=junk_bufs, name=f"junk{pi}")
                junk2 = (
                    pool.tile([P, T], F32, tag="djunk", bufs=1, name=f"junk2_{pi}")
                    if pi < m2_dve else None
                )

                # mag: m = cm - em (GPSIMD), sum m^2 (ACT or DVE)
                m = pool.tile([P, T], F32, tag="m", name=f"m{pi}")
                nc.gpsimd.tensor_tensor(m[:], cm[:], em[:], OP.subtract)
                if pi < m2_dve:
                    nc.vector.scalar_tensor_tensor(
                        junk2[:], m[:], 0.0, m[:], OP.bypass, OP.mult,
                        accum_out=acc[0:P, (c := col("m2")) : c + 1],
                    )
                else:
                    nc.scalar.activation(
                        junk[:], m[:], AF.Square,
                        accum_out=acc[0:P, (c := col("m2")) : c + 1],
                    )

                # d = cp - em; round chain -> f = q - round(q) in [-.5, .5]
                # (chunked along T so the serial chain pipelines)
                CT = T // t_chunks
                d = pool.tile([P, T], F32, tag="d", name=f"d{pi}")
                v = pool.tile([P, T], F32, tag="v", name=f"v{pi}")
                r = pool.tile([P, T], F32, tag="r", name=f"r{pi}")
                f = pool.tile([P, T], F32, tag="f", name=f"f{pi}")
                af = pool.tile([P, T], F32, tag="af", name=f"af{pi}")
                for tc0 in range(0, T, CT):
                    ts_ = slice(tc0, tc0 + CT)
                    nc.vector.tensor_tensor(d[:, ts_], cp[:, ts_], em[:, ts_], OP.subtract)
                    nc.vector.tensor_scalar(v[:, ts_], d[:, ts_], S, MAGIC, OP.mult, OP.add)
                    nc.vector.tensor_scalar_sub(r[:, ts_], v[:, ts_], MAGIC)
                    nc.vector.scalar_tensor_tensor(
                        f[:, ts_], d[:, ts_], S, r[:, ts_], OP.mult, OP.subtract
                    )
                    # ip: af = |f| acc; cspc: cos(d) = sin(pi/2 - 2pi*af) acc
                    nc.scalar.activation(
                        af[:, ts_], f[:, ts_], AF.Abs,
                        accum_out=acc[0:P, (c := col("ip")) : c + 1],
                    )
                    nc.scalar.activation(
                        junk[:, ts_], af[:, ts_], AF.Sin, bias=halfpi[0:P, :],
                        scale=NEG_TWO_PI,
                        accum_out=acc[0:P, (c := col("cos")) : c + 1],
                    )

                # gd in f-space via PE banded mm; sum dist = 0.5*N - sum ||y|-0.5|
                if f0 == 0:
                    bnd = None
                else:
                    bnd = pool.tile([1, T], F32, tag="bnd", bufs=1, name=f"bnd{pi}")
                    nc.sync.dma_start(bnd[:], f_prev[127:128, :])
                for h in range(qg_chunks):
                    qg = psum.tile([P, HT], F32, tag="qg", bufs=2, name=f"qg{pi}_{h}")
                    for n0 in range(0, HT, 512):
                        nn = h * HT + n0
                        if bnd is None:
                            nc.tensor.matmul(
                                qg[:, n0 : n0 + 512], w0[0:P, 0:P],
                                f[:, nn : nn + 512],
                            )
                        else:
                            nc.tensor.matmul(
                                qg[:, n0 : n0 + 512], w0[0:P, 0:P],
                                f[:, nn : nn + 512], start=True, stop=False,
                            )
                            nc.tensor.matmul(
                                qg[:, n0 : n0 + 512], e1s[0:1, 0:P],
                                bnd[0:1, nn : nn + 512], start=False, stop=True,
                            )
                    ag = pool.tile([P, HT], F32, tag="v", name=f"ag{pi}_{h}")
                    nc.scalar.activation(ag[:], qg[:], AF.Abs)
                    nc.scalar.activation(
                        junk[:, 0:HT], ag[:], AF.Abs, bias=neghalf[0:P, :],
                        accum_out=acc[0:P, (c := col("gd")) : c + 1],
                    )

                # iaf in f-space: fd = f[:, t-1] - f[:, t]; same dist sum
                fd = pool.tile([P, T], F32, tag="fd", name=f"fd{pi}")
                at = pool.tile([P, T], F32, tag="r", name=f"at{pi}")
                for tc0 in range(0, T, CT):
                    lo = tc0 if tc0 else 1
                    if tc0 == 0:
                        nc.vector.tensor_copy(fd[:, 0:1], f[:, 0:1])
                    nc.vector.tensor_tensor(
                        fd[:, lo : tc0 + CT], f[:, lo - 1 : tc0 + CT - 1],
                        f[:, lo : tc0 + CT], OP.subtract
                    )
                    ts_ = slice(tc0, tc0 + CT)
                    nc.scalar.activation(at[:, ts_], fd[:, ts_], AF.Abs)
                    nc.scalar.activation(
                        junk[:, ts_], at[:, ts_], AF.Abs, bias=neghalf[0:P, :],
                        accum_out=acc[0:P, (c := col("iaf")) : c + 1],
                    )
                f_prev_by_b[b] = f

            def com_pass(b, c0):
                ci = counters["ci"]
                counters["ci"] += 1
                cc = pool.tile([COM_ROWS, CK], F32, tag="com_a" if sep_com else "in_a", bufs=2 if sep_com else in_bufs, name=f"cc{ci}")
                nc.sync.dma_start(cc[:], com_c[b, :, c0 : c0 + CK])
                ec = pool.tile([COM_ROWS, CK], F32, tag="com_b" if sep_com else "in_b", bufs=2 if sep_com else in_bufs, name=f"ec{ci}")
                nc.sync.dma_start(ec[:], com_e[b, :, c0 : c0 + CK])
                cd = pool.tile([COM_ROWS, CK], F32, tag="d", name=f"cd{ci}")
                if com_dve:
                    nc.vector.tensor_tensor(cd[:], cc[:], ec[:], OP.subtract)
                    djunk = pool.tile(
                        [COM_ROWS, CK], F32, tag="junk", bufs=1, name=f"djunk{ci}"
                    )
                    nc.scalar.activation(
                        djunk[:], cd[:], AF.Square,
                        accum_out=acc[:, (c := col("c2")) : c + 1],
                    )
                else:
                    nc.gpsimd.tensor_tensor(cd[:], cc[:], ec[:], OP.subtract)
                    djunk = pool.tile(
                        [COM_ROWS, CK], F32, tag="djunk", bufs=1, name=f"djunk{ci}"
                    )
                    nc.vector.scalar_tensor_tensor(
                        djunk[:], cd[:], 0.0, cd[:], OP.bypass, OP.mult,
                        accum_out=acc[:, (c := col("c2")) : c + 1],
                    )

            def wav_pass(wi, c0):
                cw = pool.tile([WAV_ROWS, WAV_CHUNK], F32, tag="com_a" if sep_com else "in_a", bufs=2 if sep_com else in_bufs, name=f"cw{wi}")
                nc.sync.dma_start(cw[:], wav_c[:, c0 : c0 + WAV_CHUNK])
                ew = pool.tile([WAV_ROWS, WAV_CHUNK], F32, tag="com_b" if sep_com else "in_b", bufs=2 if sep_com else in_bufs, name=f"ew{wi}")
                nc.sync.dma_start(ew[:], wav_e[:, c0 : c0 + WAV_CHUNK])
                wd = pool.tile([WAV_ROWS, WAV_CHUNK], F32, tag="d", name=f"wd{wi}")
                nc.gpsimd.tensor_tensor(wd[:], cw[:], ew[:], OP.subtract)
                nc.vector.tensor_reduce(
                    acc[:, (c := col("w")) : c + 1], wd[:],
                    axis=mybir.AxisListType.X, op=OP.add, apply_absolute_value=True,
                )

            phase_list = [(b, f0, P) for b in range(BPC) for f0, P in ftiles]
            com_list = [(b, c0) for b in range(BPC) for c0 in range(0, COM_COLS, CK)]
            wav_list = list(enumerate(range(0, WAV_COLS, WAV_CHUNK)))

            if not interleave:
                for b, f0, P in phase_list:
                    phase_pass(b, f0, P)
                for b, c0 in com_list:
                    com_pass(b, c0)
                for wi, c0 in wav_list:
                    wav_pass(wi, c0)
            else:
                # weave: com chunks interspersed between phase passes
                ci, wi = 0, 0
                for k, (b, f0, P) in enumerate(phase_list):
                    if weave == 2 and ci < len(com_list):
                        com_pass(*com_list[ci])
                        ci += 1
                    phase_pass(b, f0, P)
                    for _ in range((len(com_list) + 3) // 4):
                        if ci < len(com_list):
                            com_pass(*com_list[ci])
                            ci += 1
                    if k % 2 == 1 and wi < len(wav_list):
                        wav_pass(*wav_list[wi])
                        wi += 1
                while ci < len(com_list):
                    com_pass(*com_list[ci])
                    ci += 1
                while wi < len(wav_list):
                    wav_pass(*wav_list[wi])
                    wi += 1

            # -------- final cross-partition reduce: ones^T @ acc --------
            pm = psum.tile([1, NCOLS], F32, tag="qg" if qg_chunks == 1 else "pm", bufs=2 if qg_chunks == 1 else 1)
            nc.tensor.matmul(pm[:], ones[:], acc[:])
            out_sb = pool.tile([1, NCOLS], F32, tag="out_sb", bufs=1)
            nc.vector.tensor_copy(out_sb[:], pm[:])
            nc.sync.dma_start(out_d[:], out_sb[:])

    nc.compile()
    return nc


_CACHE = {}


def _get_nc():
    if "nc" not in _CACHE:
        _CACHE["nc"] = build_nc()
    return _CACHE["nc"]


def make_in_maps(inputs):
    """Slice the full inputs into per-core input maps."""
    clean_mag = np.asarray(inputs["clean_mag"], dtype=np.float32)
    enhan_mag = np.asarray(inputs["enhan_mag"], dtype=np.float32)
    clean_pha = np.asarray(inputs["clean_pha"], dtype=np.float32)
    clean_com = np.asarray(inputs["clean_com"], dtype=np.float32)
    enhan_com = np.asarray(inputs["enhan_com"], dtype=np.float32)
    clean_wav = np.asarray(inputs["clean_wav"], dtype=np.float32)
    enhan_wav = np.asarray(inputs["enhan_wav"], dtype=np.float32)

    in_maps = []
    for i in range(NCORES):
        sl = slice(BPC * i, BPC * (i + 1))
        in_maps.append(
            {
                "mag_c": np.ascontiguousarray(clean_mag[sl]),
                "mag_e": np.ascontiguousarray(enhan_mag[sl]),
                "pha_c": np.ascontiguousarray(clean_pha[sl]),
                "com_c": np.ascontiguousarray(clean_com[sl]).reshape(
                    BPC, COM_ROWS, COM_COLS
                ),
                "com_e": np.ascontiguousarray(enhan_com[sl]).reshape(
                    BPC, COM_ROWS, COM_COLS
                ),
                "wav_c": np.ascontiguousarray(clean_wav[sl]).reshape(
                    WAV_ROWS, WAV_COLS
                ),
                "wav_e": np.ascontiguousarray(enhan_wav[sl]).reshape(
                    WAV_ROWS, WAV_COLS
                ),
            }
        )
    return in_maps


def combine(partials, inputs):
    """Combine per-core partial sums (list/array of [NCOLS]) into the 6 losses."""
    p = np.asarray(partials, dtype=np.float64).sum(axis=0)

    def tsum(term):
        return sum(p[c] for c in COLMAP[term])

    s_ip = tsum("ip")
    s_gd = tsum("gd")
    s_iaf = tsum("iaf")
    s_cos = tsum("cos")
    s_m2 = tsum("m2")
    s_c2 = tsum("c2")
    s_w = tsum("w")

    n = float(B * F * T)
    ip = TWO_PI_64 * s_ip / n
    # gd/iaf device cols hold sum(||y|-0.5|); dist(y) = 0.5 - ||y|-0.5|
    gd = TWO_PI_64 * (0.5 * n - s_gd) / n
    iaf = TWO_PI_64 * (0.5 * n - s_iaf) / n
    cspc = 1.0 - s_cos / n
    loss_mag = s_m2 / n
    loss_pha = ip + gd + iaf + cspc
    loss_com = 2.0 * s_c2 / (n * 2.0)
    loss_time = s_w / float(B * L)

    metric_g = np.asarray(inputs["metric_g"], dtype=np.float64).reshape(-1)
    one_labels = np.asarray(inputs["one_labels"], dtype=np.float64).reshape(-1)
    loss_metric = float(np.mean((metric_g - one_labels) ** 2))

    nloss = (
        loss_mag * 0.9
        + loss_pha * 0.3
        + loss_com * 0.1
        + loss_metric * 0.05
        + loss_time * 0.2
    )
    return tuple(
        np.float32(x)
        for x in (nloss, loss_mag, loss_pha, loss_com, loss_metric, loss_time)
    )


def _get_runner():
    """Build (once) a persistently-compiled 8-core sharded executor.

    Mirrors bass2jax.run_bass_via_pjrt but caches the jitted function so
    repeat calls skip retracing/recompiling. Returns
    (call(concat_inputs) -> partials[NCORES, NCOLS], in_names, device_put_fn).
    """
    if "runner" in _CACHE:
        return _CACHE["runner"]
    import jax
    from concourse import bass2jax

    nc = _get_nc()
    bass2jax.install_neuronx_cc_hook()

    partition_name = nc.partition_id_tensor.name if nc.partition_id_tensor else None
    in_names, out_names, out_avals, zero_shapes = [], [], [], []
    for alloc in nc.m.functions[0].allocations:
        if not isinstance(alloc, mybir.MemoryLocationSet):
            continue
        name = alloc.memorylocations[0].name
        if alloc.kind == "ExternalInput":
            if name != partition_name:
                in_names.append(name)
        elif alloc.kind == "ExternalOutput":
            out_names.append(name)
            shape = tuple(alloc.tensor_shape)
            dtype = mybir.dt.np(alloc.dtype)
            out_avals.append(jax.core.ShapedArray(shape, dtype))
            zero_shapes.append((shape, dtype))
    n_params = len(in_names)
    all_in = list(in_names) + list(out_names)
    if partition_name is not None:
        all_in.append(partition_name)
    donate = tuple(range(n_params, n_params + len(out_names)))

    def _body(*args):
        operands = list(args)
        if partition_name is not None:
            operands.append(bass2jax.partition_id_tensor())
        outs = bass2jax._bass_exec_p.bind(
            *operands,
            out_avals=tuple(out_avals),
            in_names=tuple(all_in),
            out_names=tuple(out_names),
            lowering_input_output_aliases=(),
            sim_require_finite=True,
            sim_require_nnan=True,
            nc=nc,
        )
        return tuple(outs)

    devices = jax.devices()[:NCORES]
    mesh = bass2jax.Mesh(np.asarray(devices), ("core",))
    pspec = bass2jax.PartitionSpec("core")
    in_specs = (pspec,) * (n_params + len(out_names))
    out_specs = (pspec,) * len(out_names)
    sharded = jax.jit(
        bass2jax.shard_map(
            _body, mesh=mesh, in_specs=in_specs, out_specs=out_specs, check_rep=False
        ),
        donate_argnums=donate,
        keep_unused=True,
    )

    def make_zeros():
        return [
            np.zeros((NCORES * s[0], *s[1:]), d) for (s, d) in zero_shapes
        ]

    def call(concat_in):
        outs = sharded(*concat_in, *make_zeros())
        return np.asarray(outs[0]).reshape(NCORES, NCOLS)

    def device_put(concat_in):
        sh = jax.sharding.NamedSharding(mesh, pspec)
        return [jax.device_put(a, sh) for a in concat_in]

    runner = (call, in_names, device_put, sharded, make_zeros)
    _CACHE["runner"] = runner
    return runner


def concat_inputs(in_maps, in_names):
    return [
        np.concatenate([m[name] for m in in_maps], axis=0) for name in in_names
    ]


def run(inputs):
    in_maps = make_in_maps(inputs)
    try:
        call, in_names, _, _, _ = _get_runner()
        partials = call(concat_inputs(in_maps, in_names))
    except Exception:
        nc = _get_nc()
        res = run_bass_kernel_spmd(nc, in_maps, core_ids=list(range(NCORES)))
        partials = [r["partials"][0] for r in res.results]
    return combine(partials, inputs)


def kernel(**inputs):
    return run(inputs)
